# revision 84
# baseline (speedup 1.0000x reference)
"""Sliding-window attention (RoPE + QKV proj + windowed softmax attention + o_proj)
for Trainium2, SPMD over 8 NeuronCores.

Sharding: batch (2) x head-groups (4 groups of 4 heads) -> 8 cores.
Each core computes qkv for its 4 heads, windowed attention, and a partial
o_proj (its heads' columns of w_o); host sums the 4 partials per batch.

Datapath is fp16 (x, w, q/k/v, probs, wo) with fp32 PSUM accumulation:
fp16 matmuls run 1 cycle/row at any moving width (f32r needs >=256), and
halve DMA traffic. Softmax normalization is folded into the P->P^T
transpose by multiplying with diag(1/rowsum) instead of the identity.
Only the diagonal and window-edge 128-blocks get a mask preload; all
other score pieces start their psum bank directly (start=True zeros the
whole 2KB bank).
"""
import sys

sys.path.insert(0, "/opt/trn_rl_repo")

import numpy as np

B = 2
S = 2048
HIDDEN = 2048
N_HEADS = 16
DH = 128
WINDOW = 512
HPC = 4  # heads per core
N_CORES = 8
QKV_O = 3 * HPC * DH  # 1536
SCALE = 1.0 / np.sqrt(DH)
NEG = -1.0e30
EXP_BIAS = -3.0  # exp(s + bias): keeps fp16 probs well inside range

_CACHE = {}

CFG = {
    "psq_bufs": 8, "psv_bufs": 2,
    "xt_extra": 14, "rope_bufs": 2, "evac_bufs": 3, "tbl_bufs": 2,
    "pm_bufs": 3, "pr_bufs": 3, "v_bufs": 32, "roll_bufs": 2,
    "sm_bufs": 8,
    "pss_bufs": 2, "pst_bufs": 2, "pso_bufs": 2,
    "psc_bufs": 8, "ob_bufs": 6,
    "phases": "ABC", "merge_bc": True,
    "spill_eng": "sync", "out_eng": "scalar",
    "pipelined": True,
}


def _enable_ldw_opt():
    """walrus crashes with --enable-ldw-opt=true; keep the default."""
    return


def _build_module(repeat=1, cfg=None):
    cfg = {**CFG, **(cfg or {})}
    import concourse.tile as tile
    from concourse import bacc, mybir
    from contextlib import ExitStack

    f32 = mybir.dt.float32
    f16 = mybir.dt.float16
    bf16 = mybir.dt.bfloat16
    AF = mybir.ActivationFunctionType

    nc = bacc.Bacc("TRN2", target_bir_lowering=False, debug=False)

    xT = nc.declare_dram_parameter("xT", [HIDDEN, S], f16, isOutput=False)
    # qkv weights pre-swizzled per 128-col output group: col t_o*HIDDEN +
    # k*128 + m holds wT[k*128+p, t_o*128+m] -> one contiguous DMA per group
    wTq = nc.declare_dram_parameter(
        "wTq", [128, 3 * HPC * HIDDEN], f16, isOutput=False
    )
    woT = nc.declare_dram_parameter("woT", [HPC * DH, HIDDEN], f16, isOutput=False)
    cosq = nc.declare_dram_parameter("cosq", [DH, S], f16, isOutput=False)
    sinq = nc.declare_dram_parameter("sinq", [DH, S], f16, isOutput=False)
    cosk = nc.declare_dram_parameter("cosk", [DH, S], f16, isOutput=False)
    sink = nc.declare_dram_parameter("sink", [DH, S], f16, isOutput=False)
    mdiag_d = nc.declare_dram_parameter("mdiag", [128, 128], bf16, isOutput=False)
    medge_d = nc.declare_dram_parameter("medge", [128, 128], bf16, isOutput=False)
    idnb_d = nc.declare_dram_parameter("idnb", [128, 128], bf16, isOutput=False)
    idn_d = nc.declare_dram_parameter("idn", [128, 128], f16, isOutput=False)
    out_d = nc.declare_dram_parameter("out", [S, HIDDEN], f16, isOutput=True)

    NKT = HIDDEN // 128  # 16 contraction chunks
    NSC = S // 512  # 4 sequence chunks
    NST = S // 128  # 16 sequence tiles

    with tile.TileContext(nc) as tc, ExitStack() as top:
        dram = top.enter_context(tc.tile_pool(name="dram", bufs=1, space="DRAM"))
        v_sdT = dram.tile([HPC * DH, S], f16, tag="vsdT")

        cpool = top.enter_context(tc.tile_pool(name="consts", bufs=1))
        mdiag = cpool.tile([128, 128], bf16, tag="mdiag")
        nc.sync.dma_start(mdiag[:], mdiag_d[:])
        medge = cpool.tile([128, 128], bf16, tag="medge")
        nc.sync.dma_start(medge[:], medge_d[:])
        idnb = cpool.tile([128, 128], bf16, tag="idnb")
        nc.sync.dma_start(idnb[:], idnb_d[:])
        idnh = cpool.tile([128, 128], f16, tag="idnh")
        nc.sync.dma_start(idnh[:], idn_d[:])
        ebias = cpool.tile([128, 1], f32, tag="ebias")
        nc.vector.memset(ebias[:], EXP_BIAS)

        attn_pool = top.enter_context(tc.tile_pool(name="attn", bufs=HPC))
        # rope'd q/k stay in SBUF between phases A and B; double-buffered
        # (2 reps x 8 tiles) so repeat n+1's phase A never waits on repeat
        # n's phase B reads. v blocks reload via DMA-transpose during A.
        qkv_pool = top.enter_context(tc.tile_pool(name="qkv", bufs=4 * HPC))
        v_pool = top.enter_context(tc.tile_pool(name="vt", bufs=CFG["v_bufs"]))

        for rep in range(repeat):
            qk_tiles = [
                qkv_pool.tile([128, S], f16, tag="qk", name=f"qk{rep}_{t}")
                for t in range(2 * HPC)
            ]
            vblocks = [
                v_pool.tile([128, HPC * DH], f16, tag="vb", name=f"vb{rep}_{j}")
                for j in range(NST)
            ]
            # ------------- Phase A: QKV projection + RoPE -------------
            if "A" in cfg["phases"]:
              with ExitStack() as ph:
                wt_pool = ph.enter_context(tc.tile_pool(name="wt", bufs=NKT))
                xt_pool = ph.enter_context(tc.tile_pool(name="xt", bufs=NKT + cfg["xt_extra"]))
                tbl_pool = ph.enter_context(tc.tile_pool(name="tbl", bufs=cfg["tbl_bufs"]))
                rope_pool = ph.enter_context(tc.tile_pool(name="rope", bufs=cfg["rope_bufs"]))
                evac_pool = ph.enter_context(tc.tile_pool(name="evac", bufs=cfg["evac_bufs"]))
                psq_pool = ph.enter_context(
                    tc.tile_pool(name="psq", bufs=cfg["psq_bufs"], space="PSUM")
                )

                # q/k weights as [128,128] column tiles, group-major, so the
                # first projection group's weights land in ~2us instead of
                # waiting for full [128, QKV_O] rows; v weights as 512-wide
                # moving tiles. xt goes on the Activation hwdge queue, wt on
                # the SP queue -- the two streams run in parallel.
                # All loads go on the SP hwdge queue: the Activation queue
                # must stay empty so phase B's exp chain isn't serialized
                # behind load dispatches (engine queues are in-order).
                # Emission order puts group 0's inputs first.
                def load_wtq(t_o):
                    t = wt_pool.tile(
                        [128, HIDDEN], f16, tag="wtq", bufs=3 * HPC
                    )
                    nc.sync.dma_start(
                        t[:], wTq[:, t_o * HIDDEN : (t_o + 1) * HIDDEN]
                    )
                    return t

                # group 0's weights in 4 chunks interleaved with the first
                # xt tiles: subtile deps let the k-loop start after ~1 chunk
                wtq0 = wt_pool.tile([128, HIDDEN], f16, tag="wtq", bufs=3 * HPC)
                wtq = [wtq0]

                def load_xt(sc, interleave_wtq0=False):
                    s0 = sc * 512
                    xt_tiles = []
                    for k in range(NKT):
                        if interleave_wtq0 and k % 4 == 0:
                            c = k // 4
                            nc.sync.dma_start(
                                wtq0[:, c * 512 : (c + 1) * 512],
                                wTq[:, c * 512 : (c + 1) * 512],
                            )
                        t = xt_pool.tile([128, 512], f16, tag="xt")
                        nc.sync.dma_start(
                            t[:], xT[k * 128 : (k + 1) * 128, s0 : s0 + 512]
                        )
                        xt_tiles.append(t)
                    return xt_tiles

                def load_tbl(sc):
                    s0 = sc * 512
                    tb = {}
                    for nm, src in (
                        ("cosq", cosq),
                        ("sinq", sinq),
                        ("cosk", cosk),
                        ("sink", sink),
                    ):
                        t = tbl_pool.tile([128, 512], f16, tag=nm)
                        nc.sync.dma_start(t[:], src[:, s0 : s0 + 512])
                        tb[nm] = t
                    return tb

                def load_xt_tbl(sc):
                    return load_xt(sc), load_tbl(sc)

                # arrival deadlines: group g's weights at ~3.4us*g, all xt
                # within group 0, tables only by the first rope -> order
                # wtq0, xt, wtq1, tbl, rest
                xt0 = load_xt(0, interleave_wtq0=True)
                wtq.append(load_wtq(1))
                tb0 = load_tbl(0)
                wtq += [load_wtq(t_o) for t_o in range(2, 3 * HPC)]

                for sc in range(NSC):
                    s0 = sc * 512
                    xt_tiles, tb = (xt0, tb0) if sc == 0 else load_xt_tbl(sc)

                    def rope_evac(t_o, ps):
                        ct = tb["cosq"] if t_o < HPC else tb["cosk"]
                        st = tb["sinq"] if t_o < HPC else tb["sink"]
                        tmp = rope_pool.tile([128, 512], f16, tag="tmp")
                        nc.vector.tensor_mul(tmp[0:64, :], ps[64:128, :], st[0:64, :])
                        nc.vector.tensor_mul(
                            tmp[64:128, :], ps[0:64, :], st[64:128, :]
                        )
                        qc = rope_pool.tile([128, 512], f16, tag="qc")
                        nc.vector.tensor_mul(qc[:], ps[:], ct[:])
                        nc.vector.tensor_add(
                            qk_tiles[t_o][:, s0 : s0 + 512], qc[:], tmp[:]
                        )

                    def v_evac(vg, psv):
                        # v in [vcol, seq] layout; spill then reload each
                        # 128-seq block via DMA-transpose as [seq, vcol]
                        vo = evac_pool.tile([128, 512], f16, tag="vo")
                        nc.vector.tensor_copy(vo[:], psv[:])
                        getattr(nc, cfg["spill_eng"]).dma_start(
                            v_sdT[vg * 128 : (vg + 1) * 128, s0 : s0 + 512],
                            vo[:],
                        )

                    for t_o in range(3 * HPC):
                        ps = psq_pool.tile([128, 512], f32, tag="psq")
                        for k in range(NKT):
                            nc.tensor.matmul(
                                ps[:],
                                wtq[t_o][:, k * 128 : (k + 1) * 128],
                                xt_tiles[k][:],
                                start=(k == 0),
                                stop=(k == NKT - 1),
                            )
                        if t_o < 2 * HPC:
                            rope_evac(t_o, ps)
                        else:
                            v_evac(t_o - 2 * HPC, ps)
                    for jb in range(4 * sc, 4 * sc + 4):
                        nc.sync.dma_start_transpose(
                            vblocks[jb][:],
                            v_sdT[:, jb * 128 : (jb + 1) * 128],
                        )

            # ------------- Phase C body (emitted merged or standalone) ----
            attn_tiles = []

            def emit_phase_c(ph):
                ob_pool = ph.enter_context(
                    tc.tile_pool(name="ob", bufs=cfg["ob_bufs"])
                )
                psc_pool = ph.enter_context(
                    tc.tile_pool(name="psc", bufs=cfg["psc_bufs"], space="PSUM")
                )
                wo_pool = ph.enter_context(tc.tile_pool(name="wo", bufs=HPC))
                wts = []
                for h in range(HPC):
                    t = wo_pool.tile([128, HIDDEN], f16, tag="wo")
                    nc.sync.dma_start(t[:], woT[h * 128 : (h + 1) * 128, :])
                    wts.append(t)
                # h middle / mc inner: the stationary (attn piece) is reused
                # across the 4 mc matmuls, so the hardware reloads weights
                # once per (st_i, h) instead of per matmul.
                for st_i in range(NST):
                    pss4 = [
                        psc_pool.tile([128, 512], f32, tag="psc", name=f"psc{mc}")
                        for mc in range(HIDDEN // 512)
                    ]
                    for h in range(HPC):
                        for mc in range(HIDDEN // 512):
                            nc.tensor.matmul(
                                pss4[mc][:],
                                attn_tiles[h][:, st_i * 128 : (st_i + 1) * 128],
                                wts[h][:, mc * 512 : (mc + 1) * 512],
                                start=(h == 0),
                                stop=(h == HPC - 1),
                            )
                    for mc in range(HIDDEN // 512):
                        ob = ob_pool.tile([128, 512], f16, tag="ob")
                        nc.scalar.activation(ob[:], pss4[mc][:], AF.Copy)
                        getattr(nc, cfg["out_eng"]).dma_start(
                            out_d[
                                st_i * 128 : (st_i + 1) * 128,
                                mc * 512 : (mc + 1) * 512,
                            ],
                            ob[:],
                        )

            # ------------- Phase B: windowed attention -------------
            if "B" in cfg["phases"]:
              with ExitStack() as ph:
                pm_pool = ph.enter_context(tc.tile_pool(name="pm", bufs=cfg["pm_bufs"]))
                pr_pool = ph.enter_context(tc.tile_pool(name="pr", bufs=cfg["pr_bufs"]))
                sm_pool = ph.enter_context(tc.tile_pool(name="sm", bufs=cfg["sm_bufs"]))
                strip_pool = ph.enter_context(
                    tc.tile_pool(name="strip", bufs=cfg["roll_bufs"])
                )
                phps = ExitStack()
                pss_pool = phps.enter_context(
                    tc.tile_pool(name="pss", bufs=cfg["pss_bufs"], space="PSUM")
                )
                pst_pool = phps.enter_context(
                    tc.tile_pool(name="pst", bufs=cfg["pst_bufs"], space="PSUM")
                )
                pso_pool = phps.enter_context(
                    tc.tile_pool(name="pso", bufs=cfg["pso_bufs"], space="PSUM")
                )

                # last q-block whose PV contribution lands in psum bank bk
                LAST_BANK = {0: 3, 1: 7, 2: 11, 3: 15}

                def make_pieces():
                    """PV pieces keyed by ready block, with start/stop flags.

                    Pieces are cut at psum bank boundaries and at each bank's
                    high-water mark so a single matmul never mixes
                    already-written and pending-zero bytes. The first piece
                    per bank carries start=True (zeros the whole bank)."""
                    hw = {}
                    by_ready = {}
                    for jb in range(NST):
                        w0, w1 = jb * 128, min(jb * 128 + 640, S)
                        c = w0
                        while c < w1:
                            bk = c // 512
                            nxt = min(w1, (bk + 1) * 512)
                            first = bk not in hw
                            hm = hw.get(bk, c)
                            if not first and c < hm < nxt:
                                subs = [(c, hm), (hm, nxt)]
                            else:
                                subs = [(c, nxt)]
                            for a, b in subs:
                                by_ready.setdefault(min(jb + 4, NST - 1), []).append(
                                    (
                                        jb,
                                        a,
                                        b,
                                        first and a == c,
                                        jb == LAST_BANK[bk] and b == nxt,
                                    )
                                )
                            hw[bk] = max(hm, nxt)
                            c = nxt
                    return by_ready

                PIECES = make_pieces()

                def setup_head(h):
                    qh = qk_tiles[h]
                    kh = qk_tiles[HPC + h]
                    vt = [vblocks[jb][:, h * 128 : (h + 1) * 128] for jb in range(NST)]
                    ah = attn_pool.tile([128, S], f16, tag="ah", name=f"ah{h}")
                    attn_tiles.append(ah)
                    roll = strip_pool.tile(
                        [128, 8 * 640], f16, tag="roll", name=f"roll{h}"
                    )
                    return dict(
                        h=h, qh=qh, kh=kh, vt=vt, ah=ah,
                        pv_banks=[None] * 4, roll=roll,
                    )

                def strip_ap(st, jb, c0, c1):
                    base = (jb % 8) * 640
                    return st["roll"][:, base + c0 : base + c1]

                def emit_scores(st, i):
                    """Scores + mask + exp + 1/rowsum diag for block i."""
                    qh, kh = st["qh"], st["kh"]
                    jlo = max(0, i * 128 - WINDOW)
                    w = i * 128 + 128 - jlo
                    qb = qh[:, i * 128 : (i + 1) * 128]
                    ps_s = pss_pool.tile([128, 1024], f32, tag="pss")
                    if i >= 4:
                        # bank 0: edge-mask preload + 128-wide edge scores,
                        # then the 384-wide unmasked middle (pending-zero)
                        nc.tensor.matmul(
                            ps_s[:, 0:128], idnb[:], medge[:],
                            start=True, stop=False, skip_group_check=True,
                        )
                        nc.tensor.matmul(
                            ps_s[:, 128:512],
                            qb, kh[:, jlo + 128 : jlo + 512],
                            start=False, stop=False, skip_group_check=True,
                        )
                        nc.tensor.matmul(
                            ps_s[:, 0:128],
                            qb, kh[:, jlo : jlo + 128],
                            start=False, stop=True, skip_group_check=True,
                        )
                        # bank 1: diagonal block
                        nc.tensor.matmul(
                            ps_s[:, 512:640], idnb[:], mdiag[:],
                            start=True, stop=False, skip_group_check=True,
                        )
                        nc.tensor.matmul(
                            ps_s[:, 512:640],
                            qb, kh[:, jlo + 512 : jlo + 640],
                            start=False, stop=True, skip_group_check=True,
                        )
                    else:
                        dcol = w - 128
                        nc.tensor.matmul(
                            ps_s[:, dcol:w], idnb[:], mdiag[:],
                            start=True, stop=False, skip_group_check=True,
                        )
                        if dcol > 0:
                            nc.tensor.matmul(
                                ps_s[:, 0:dcol],
                                qb, kh[:, 0:dcol],
                                start=False, stop=False, skip_group_check=True,
                            )
                        nc.tensor.matmul(
                            ps_s[:, dcol:w],
                            qb, kh[:, dcol:w],
                            start=False, stop=True, skip_group_check=True,
                        )
                    pm = pm_pool.tile([128, 640], f16, tag="pm")
                    sums = sm_pool.tile([128, 1], f32, tag="sums")
                    nc.scalar.activation(
                        pm[:, :w], ps_s[:, :w], AF.Exp,
                        bias=ebias[:], accum_out=sums[:],
                    )
                    rc = sm_pool.tile([128, 1], f32, tag="rc")
                    nc.vector.reciprocal(rc[:], sums[:])
                    # normalize in fp16 (2-byte packed operands -> 2x DVE)
                    pr = pr_pool.tile([128, 640], f16, tag="pr")
                    nc.vector.tensor_scalar_mul(pr[:, :w], pm[:, :w], rc[:])
                    st["pms"][i] = pr

                def emit_tail(st, i):
                    """Transpose of normalized probs into strips + PV for i."""
                    h, pr = st["h"], st["pms"][i]
                    jlo = max(0, i * 128 - WINDOW)
                    w = i * 128 + 128 - jlo
                    nblk = w // 128
                    j0 = jlo // 128
                    # PE transpose with f16 psum output (1 bank for 640 cols)
                    ps_t = pst_pool.tile([128, 1024], f16, tag="pst")
                    for z in range(nblk):
                        nc.tensor.matmul(
                            ps_t[:, z * 128 : (z + 1) * 128],
                            pr[:, z * 128 : (z + 1) * 128],
                            idnh[:],
                            is_transpose=True,
                            start=(z == 0),
                            stop=(z == nblk - 1),
                            skip_group_check=True,
                        )
                    roll = st["roll"]
                    # dest col for z: ((j0+z)%8)*640 + (i-j0-z)*128 advances by
                    # 512 per z within a non-wrapping slot segment -> axis-
                    # aligned in a (col/512, (col%512)/128) view.
                    roll4 = roll[:].rearrange("p (a b c) -> p a b c", b=4, c=128)
                    ps4 = ps_t[:, : nblk * 128].rearrange(
                        "p (z o c) -> p z o c", o=1, c=128
                    )
                    z = 0
                    while z < nblk:
                        s0 = (j0 + z) % 8
                        zlen = min(nblk - z, 8 - s0)
                        base = s0 * 640 + (i - j0 - z) * 128
                        a0, b0 = base // 512, (base % 512) // 128
                        nc.vector.tensor_copy(
                            roll4[:, a0 : a0 + zlen, b0 : b0 + 1, :],
                            ps4[:, z : z + zlen, :, :],
                        )
                        z += zlen
                    # PV pieces that become ready once block i's strips exist
                    vt, ah = st["vt"], st["ah"]
                    pv_banks = st["pv_banks"]
                    for jb, c, nxt, pstart, pstop in PIECES.get(i, ()):
                        bk = c // 512
                        if pv_banks[bk] is None:
                            pv_banks[bk] = pso_pool.tile(
                                [128, 512], f32, tag="pvo",
                                name=f"pvo_h{h}_b{bk}",
                            )
                        nc.tensor.matmul(
                            pv_banks[bk][:, c - bk * 512 : nxt - bk * 512],
                            vt[jb],
                            strip_ap(st, jb, c - jb * 128, nxt - jb * 128),
                            start=pstart,
                            stop=pstop,
                            skip_group_check=True,
                        )
                        if pstop:
                            nc.vector.tensor_copy(
                                ah[:, bk * 512 : (bk + 1) * 512],
                                pv_banks[bk][:],
                            )

                for h in range(HPC):
                    st = setup_head(h)
                    st["pms"] = {}
                    if cfg["pipelined"]:
                        for i in range(NST):
                            emit_scores(st, i)
                            if i > 0:
                                emit_tail(st, i - 1)
                        emit_tail(st, NST - 1)
                    else:
                        for i in range(NST):
                            emit_scores(st, i)
                            emit_tail(st, i)

                phps.close()
                if cfg["merge_bc"] and "C" in cfg["phases"]:
                    emit_phase_c(ph)

            # ------------- Phase C: output projection (standalone) -------------
            if "C" in cfg["phases"] and not cfg["merge_bc"]:
                with ExitStack() as ph:
                    emit_phase_c(ph)

    nc.compile()
    return nc


def _get_module(repeat=1, cfg=None):
    _enable_ldw_opt()
    key = ("nc", repeat, tuple(sorted((cfg or {}).items())))
    if key not in _CACHE:
        _CACHE[key] = _build_module(repeat, cfg)
    return _CACHE[key]


def make_in_maps(hidden_states, cos, sin, w_qkv, w_o):
    hidden_states = np.asarray(hidden_states, dtype=np.float32)
    cos = np.asarray(cos, dtype=np.float32)
    sin = np.asarray(sin, dtype=np.float32)
    w_qkv = np.asarray(w_qkv, dtype=np.float32)
    w_o = np.asarray(w_o, dtype=np.float32)

    cosT = np.ascontiguousarray(cos.T)  # [DH, S]
    sinT = np.ascontiguousarray(sin.T)
    sinS = sinT.copy()
    sinS[: DH // 2] *= -1.0  # fold rotate_half sign
    cq = (cosT * SCALE).astype(np.float16)
    sq = (sinS * SCALE).astype(np.float16)
    ck = cosT.astype(np.float16)
    sk = sinS.astype(np.float16)

    import ml_dtypes

    rr = np.arange(128)[:, None]
    cc = np.arange(128)[None, :]
    # diagonal block (jb == i): allow j <= q  <->  c <= r
    mdiag = np.where(cc <= rr, 0.0, NEG).astype(ml_dtypes.bfloat16)
    # window-edge block (jb == i-4): allow q-j < 512  <->  c > r
    medge = np.where(cc > rr, 0.0, NEG).astype(ml_dtypes.bfloat16)
    idn = np.eye(128, dtype=np.float16)
    idnb = np.eye(128, dtype=ml_dtypes.bfloat16)

    xTs = [np.ascontiguousarray(hidden_states[b].T).astype(np.float16) for b in range(B)]

    in_maps = []
    for c in range(N_CORES):
        b, hg = divmod(c, N_CORES // B)
        r0 = hg * HPC * DH
        wq = w_qkv[r0 : r0 + HPC * DH]
        wk = w_qkv[N_HEADS * DH + r0 : N_HEADS * DH + r0 + HPC * DH]
        wv = w_qkv[2 * N_HEADS * DH + r0 : 2 * N_HEADS * DH + r0 + HPC * DH]
        wTqk = np.concatenate([wq, wk, wv], axis=0).T.astype(np.float16)  # [2048, 1536]
        # swizzle: wTq_sw[p, t_o*HIDDEN + k*128 + m] = wTqk[k*128+p, t_o*128+m]
        w4 = wTqk.reshape(HIDDEN // 128, 128, 3 * HPC, DH)  # [k, p, t_o, m]
        wTq_sw = np.ascontiguousarray(
            w4.transpose(1, 2, 0, 3).reshape(128, 3 * HPC * HIDDEN)
        )
        woTc = np.ascontiguousarray(w_o[:, r0 : r0 + HPC * DH].T).astype(np.float16)
        in_maps.append(
            {
                "xT": xTs[b],
                "wTq": wTq_sw,
                "woT": woTc,
                "cosq": cq,
                "sinq": sq,
                "cosk": ck,
                "sink": sk,
                "mdiag": mdiag,
                "medge": medge,
                "idn": idn,
                "idnb": idnb,
            }
        )
    return in_maps


def gather(results):
    out = np.zeros((B, S, HIDDEN), dtype=np.float32)
    for c in range(N_CORES):
        b = c // (N_CORES // B)
        out[b] += results[c]["out"].astype(np.float32)
    return out


def kernel(hidden_states, cos, sin, w_qkv, w_o):
    from concourse.bass_utils import run_bass_kernel_spmd

    nc = _get_module()
    in_maps = make_in_maps(hidden_states, cos, sin, w_qkv, w_o)
    res = run_bass_kernel_spmd(nc, in_maps, list(range(N_CORES)))
    return gather(res.results)


# revision 87
# speedup vs baseline: 1.0414x; 1.0414x over previous
"""Sliding-window attention (RoPE + QKV proj + windowed softmax attention + o_proj)
for Trainium2, SPMD over 8 NeuronCores.

Sharding: batch (2) x head-groups (4 groups of 4 heads) -> 8 cores.
Each core computes qkv for its 4 heads, windowed attention, and a partial
o_proj (its heads' columns of w_o); host sums the 4 partials per batch.

Datapath is fp16 (x, w, q/k/v, probs, wo) with fp32 PSUM accumulation:
fp16 matmuls run 1 cycle/row at any moving width (f32r needs >=256), and
halve DMA traffic. Softmax normalization is folded into the P->P^T
transpose by multiplying with diag(1/rowsum) instead of the identity.
Only the diagonal and window-edge 128-blocks get a mask preload; all
other score pieces start their psum bank directly (start=True zeros the
whole 2KB bank).
"""
import sys

sys.path.insert(0, "/opt/trn_rl_repo")

import numpy as np

B = 2
S = 2048
HIDDEN = 2048
N_HEADS = 16
DH = 128
WINDOW = 512
HPC = 4  # heads per core
N_CORES = 8
QKV_O = 3 * HPC * DH  # 1536
SCALE = 1.0 / np.sqrt(DH)
NEG = -1.0e30
EXP_BIAS = -3.0  # exp(s + bias): keeps fp16 probs well inside range

_CACHE = {}

CFG = {
    "psq_bufs": 8, "psv_bufs": 2,
    "xt_extra": 14, "rope_bufs": 2, "evac_bufs": 3, "tbl_bufs": 2,
    "pm_bufs": 3, "pr_bufs": 3, "v_bufs": 32, "roll_bufs": 2,
    "sm_bufs": 8,
    "pss_bufs": 2, "pst_bufs": 2, "pso_bufs": 2,
    "psc_bufs": 4, "ob_bufs": 6,
    "phases": "ABC", "merge_bc": True,
    "spill_eng": "sync", "out_eng": "scalar",
    "pipelined": True,
}


def _enable_ldw_opt():
    """walrus crashes with --enable-ldw-opt=true; keep the default."""
    return


def _build_module(repeat=1, cfg=None):
    cfg = {**CFG, **(cfg or {})}
    import concourse.tile as tile
    from concourse import bacc, mybir
    from contextlib import ExitStack

    f32 = mybir.dt.float32
    f16 = mybir.dt.float16
    bf16 = mybir.dt.bfloat16
    AF = mybir.ActivationFunctionType

    nc = bacc.Bacc("TRN2", target_bir_lowering=False, debug=False)

    xT = nc.declare_dram_parameter("xT", [HIDDEN, S], f16, isOutput=False)
    # qkv weights pre-swizzled per 128-col output group: col t_o*HIDDEN +
    # k*128 + m holds wT[k*128+p, t_o*128+m] -> one contiguous DMA per group
    wTq = nc.declare_dram_parameter(
        "wTq", [128, 3 * HPC * HIDDEN], f16, isOutput=False
    )
    woT = nc.declare_dram_parameter("woT", [HPC * DH, HIDDEN], f16, isOutput=False)
    cosq = nc.declare_dram_parameter("cosq", [DH, S], f16, isOutput=False)
    sinq = nc.declare_dram_parameter("sinq", [DH, S], f16, isOutput=False)
    cosk = nc.declare_dram_parameter("cosk", [DH, S], f16, isOutput=False)
    sink = nc.declare_dram_parameter("sink", [DH, S], f16, isOutput=False)
    mdiag_d = nc.declare_dram_parameter("mdiag", [128, 128], bf16, isOutput=False)
    medge_d = nc.declare_dram_parameter("medge", [128, 128], bf16, isOutput=False)
    idnb_d = nc.declare_dram_parameter("idnb", [128, 128], bf16, isOutput=False)
    idn_d = nc.declare_dram_parameter("idn", [128, 128], f16, isOutput=False)
    out_d = nc.declare_dram_parameter("out", [S, HIDDEN], f16, isOutput=True)

    NKT = HIDDEN // 128  # 16 contraction chunks
    NSC = S // 512  # 4 sequence chunks
    NST = S // 128  # 16 sequence tiles

    with tile.TileContext(nc) as tc, ExitStack() as top:
        dram = top.enter_context(tc.tile_pool(name="dram", bufs=1, space="DRAM"))
        v_sdT = dram.tile([HPC * DH, S], f16, tag="vsdT")

        cpool = top.enter_context(tc.tile_pool(name="consts", bufs=1))
        mdiag = cpool.tile([128, 128], bf16, tag="mdiag")
        nc.sync.dma_start(mdiag[:], mdiag_d[:])
        medge = cpool.tile([128, 128], bf16, tag="medge")
        nc.sync.dma_start(medge[:], medge_d[:])
        idnb = cpool.tile([128, 128], bf16, tag="idnb")
        nc.sync.dma_start(idnb[:], idnb_d[:])
        idnh = cpool.tile([128, 128], f16, tag="idnh")
        nc.sync.dma_start(idnh[:], idn_d[:])
        ebias = cpool.tile([128, 1], f32, tag="ebias")
        nc.vector.memset(ebias[:], EXP_BIAS)

        attn_pool = top.enter_context(tc.tile_pool(name="attn", bufs=HPC))
        # rope'd q/k stay in SBUF between phases A and B; double-buffered
        # (2 reps x 8 tiles) so repeat n+1's phase A never waits on repeat
        # n's phase B reads. v blocks reload via DMA-transpose during A.
        qkv_pool = top.enter_context(tc.tile_pool(name="qkv", bufs=4 * HPC))
        v_pool = top.enter_context(tc.tile_pool(name="vt", bufs=CFG["v_bufs"]))

        for rep in range(repeat):
            qk_tiles = [
                qkv_pool.tile([128, S], f16, tag="qk", name=f"qk{rep}_{t}")
                for t in range(2 * HPC)
            ]
            vblocks = [
                v_pool.tile([128, HPC * DH], f16, tag="vb", name=f"vb{rep}_{j}")
                for j in range(NST)
            ]
            # ------------- Phase A: QKV projection + RoPE -------------
            if "A" in cfg["phases"]:
              with ExitStack() as ph:
                wt_pool = ph.enter_context(tc.tile_pool(name="wt", bufs=NKT))
                xt_pool = ph.enter_context(tc.tile_pool(name="xt", bufs=NKT + cfg["xt_extra"]))
                tbl_pool = ph.enter_context(tc.tile_pool(name="tbl", bufs=cfg["tbl_bufs"]))
                rope_pool = ph.enter_context(tc.tile_pool(name="rope", bufs=cfg["rope_bufs"]))
                evac_pool = ph.enter_context(tc.tile_pool(name="evac", bufs=cfg["evac_bufs"]))
                psq_pool = ph.enter_context(
                    tc.tile_pool(name="psq", bufs=cfg["psq_bufs"], space="PSUM")
                )

                # q/k weights as [128,128] column tiles, group-major, so the
                # first projection group's weights land in ~2us instead of
                # waiting for full [128, QKV_O] rows; v weights as 512-wide
                # moving tiles. xt goes on the Activation hwdge queue, wt on
                # the SP queue -- the two streams run in parallel.
                # All loads go on the SP hwdge queue: the Activation queue
                # must stay empty so phase B's exp chain isn't serialized
                # behind load dispatches (engine queues are in-order).
                # Emission order puts group 0's inputs first.
                def load_wtq(t_o):
                    t = wt_pool.tile(
                        [128, HIDDEN], f16, tag="wtq", bufs=3 * HPC
                    )
                    nc.sync.dma_start(
                        t[:], wTq[:, t_o * HIDDEN : (t_o + 1) * HIDDEN]
                    )
                    return t

                wtq = [load_wtq(0)]

                def load_xt(sc):
                    s0 = sc * 512
                    xt_tiles = []
                    for k in range(NKT):
                        t = xt_pool.tile([128, 512], f16, tag="xt")
                        nc.sync.dma_start(
                            t[:], xT[k * 128 : (k + 1) * 128, s0 : s0 + 512]
                        )
                        xt_tiles.append(t)
                    return xt_tiles

                def load_tbl(sc):
                    s0 = sc * 512
                    tb = {}
                    for nm, src in (
                        ("cosq", cosq),
                        ("sinq", sinq),
                        ("cosk", cosk),
                        ("sink", sink),
                    ):
                        t = tbl_pool.tile([128, 512], f16, tag=nm)
                        nc.sync.dma_start(t[:], src[:, s0 : s0 + 512])
                        tb[nm] = t
                    return tb

                def load_xt_tbl(sc):
                    return load_xt(sc), load_tbl(sc)

                # arrival deadlines: group g's weights at ~3.4us*g, all xt
                # within group 0, tables only by the first rope -> order
                # wtq0, xt, wtq1, tbl, rest
                xt0 = load_xt(0)
                wtq.append(load_wtq(1))
                tb0 = load_tbl(0)
                wtq += [load_wtq(t_o) for t_o in range(2, 3 * HPC)]

                for sc in range(NSC):
                    s0 = sc * 512
                    xt_tiles, tb = (xt0, tb0) if sc == 0 else load_xt_tbl(sc)

                    def rope_evac(t_o, ps):
                        ct = tb["cosq"] if t_o < HPC else tb["cosk"]
                        st = tb["sinq"] if t_o < HPC else tb["sink"]
                        tmp = rope_pool.tile([128, 512], f16, tag="tmp")
                        nc.vector.tensor_mul(tmp[0:64, :], ps[64:128, :], st[0:64, :])
                        nc.vector.tensor_mul(
                            tmp[64:128, :], ps[0:64, :], st[64:128, :]
                        )
                        qc = rope_pool.tile([128, 512], f16, tag="qc")
                        nc.vector.tensor_mul(qc[:], ps[:], ct[:])
                        nc.vector.tensor_add(
                            qk_tiles[t_o][:, s0 : s0 + 512], qc[:], tmp[:]
                        )

                    def v_evac(vg, psv):
                        # v in [vcol, seq] layout; spill then reload each
                        # 128-seq block via DMA-transpose as [seq, vcol]
                        vo = evac_pool.tile([128, 512], f16, tag="vo")
                        nc.vector.tensor_copy(vo[:], psv[:])
                        getattr(nc, cfg["spill_eng"]).dma_start(
                            v_sdT[vg * 128 : (vg + 1) * 128, s0 : s0 + 512],
                            vo[:],
                        )

                    for t_o in range(3 * HPC):
                        ps = psq_pool.tile([128, 512], f32, tag="psq")
                        for k in range(NKT):
                            nc.tensor.matmul(
                                ps[:],
                                wtq[t_o][:, k * 128 : (k + 1) * 128],
                                xt_tiles[k][:],
                                start=(k == 0),
                                stop=(k == NKT - 1),
                            )
                        if t_o < 2 * HPC:
                            rope_evac(t_o, ps)
                        else:
                            v_evac(t_o - 2 * HPC, ps)
                    for jb in range(4 * sc, 4 * sc + 4):
                        nc.sync.dma_start_transpose(
                            vblocks[jb][:],
                            v_sdT[:, jb * 128 : (jb + 1) * 128],
                        )

            # ------------- Phase C body (emitted merged or standalone) ----
            attn_tiles = []

            def emit_phase_c(ph):
                ob_pool = ph.enter_context(
                    tc.tile_pool(name="ob", bufs=cfg["ob_bufs"])
                )
                psc_pool = ph.enter_context(
                    tc.tile_pool(name="psc", bufs=cfg["psc_bufs"], space="PSUM")
                )
                wo_pool = ph.enter_context(tc.tile_pool(name="wo", bufs=HPC))
                wts = []
                for h in range(HPC):
                    t = wo_pool.tile([128, HIDDEN], f16, tag="wo")
                    nc.sync.dma_start(t[:], woT[h * 128 : (h + 1) * 128, :])
                    wts.append(t)
                # h middle / mc inner: the stationary (attn piece) is reused
                # across the 4 mc matmuls, so the hardware reloads weights
                # once per (st_i, h) instead of per matmul.
                for st_i in range(NST):
                    pss4 = [
                        psc_pool.tile([128, 512], f32, tag="psc", name=f"psc{mc}")
                        for mc in range(HIDDEN // 512)
                    ]
                    for h in range(HPC):
                        for mc in range(HIDDEN // 512):
                            nc.tensor.matmul(
                                pss4[mc][:],
                                attn_tiles[h][:, st_i * 128 : (st_i + 1) * 128],
                                wts[h][:, mc * 512 : (mc + 1) * 512],
                                start=(h == 0),
                                stop=(h == HPC - 1),
                            )
                    for mc in range(HIDDEN // 512):
                        ob = ob_pool.tile([128, 512], f16, tag="ob")
                        nc.scalar.activation(ob[:], pss4[mc][:], AF.Copy)
                        getattr(nc, cfg["out_eng"]).dma_start(
                            out_d[
                                st_i * 128 : (st_i + 1) * 128,
                                mc * 512 : (mc + 1) * 512,
                            ],
                            ob[:],
                        )

            # ------------- Phase B: windowed attention -------------
            if "B" in cfg["phases"]:
              with ExitStack() as ph:
                pm_pool = ph.enter_context(tc.tile_pool(name="pm", bufs=cfg["pm_bufs"]))
                pr_pool = ph.enter_context(tc.tile_pool(name="pr", bufs=cfg["pr_bufs"]))
                sm_pool = ph.enter_context(tc.tile_pool(name="sm", bufs=cfg["sm_bufs"]))
                strip_pool = ph.enter_context(
                    tc.tile_pool(name="strip", bufs=cfg["roll_bufs"])
                )
                phps = ExitStack()
                pss_pool = phps.enter_context(
                    tc.tile_pool(name="pss", bufs=cfg["pss_bufs"], space="PSUM")
                )
                pst_pool = phps.enter_context(
                    tc.tile_pool(name="pst", bufs=cfg["pst_bufs"], space="PSUM")
                )
                pso_pool = phps.enter_context(
                    tc.tile_pool(name="pso", bufs=cfg["pso_bufs"], space="PSUM")
                )

                # last q-block whose PV contribution lands in psum bank bk
                LAST_BANK = {0: 3, 1: 7, 2: 11, 3: 15}

                def make_pieces():
                    """PV pieces keyed by ready block, with start/stop flags.

                    Pieces are cut at psum bank boundaries and at each bank's
                    high-water mark so a single matmul never mixes
                    already-written and pending-zero bytes. The first piece
                    per bank carries start=True (zeros the whole bank)."""
                    hw = {}
                    by_ready = {}
                    for jb in range(NST):
                        w0, w1 = jb * 128, min(jb * 128 + 640, S)
                        c = w0
                        while c < w1:
                            bk = c // 512
                            nxt = min(w1, (bk + 1) * 512)
                            first = bk not in hw
                            hm = hw.get(bk, c)
                            if not first and c < hm < nxt:
                                subs = [(c, hm), (hm, nxt)]
                            else:
                                subs = [(c, nxt)]
                            for a, b in subs:
                                by_ready.setdefault(min(jb + 4, NST - 1), []).append(
                                    (
                                        jb,
                                        a,
                                        b,
                                        first and a == c,
                                        jb == LAST_BANK[bk] and b == nxt,
                                    )
                                )
                            hw[bk] = max(hm, nxt)
                            c = nxt
                    return by_ready

                PIECES = make_pieces()

                def setup_head(h):
                    qh = qk_tiles[h]
                    kh = qk_tiles[HPC + h]
                    vt = [vblocks[jb][:, h * 128 : (h + 1) * 128] for jb in range(NST)]
                    ah = attn_pool.tile([128, S], f16, tag="ah", name=f"ah{h}")
                    attn_tiles.append(ah)
                    roll = strip_pool.tile(
                        [128, 8 * 640], f16, tag="roll", name=f"roll{h}"
                    )
                    return dict(
                        h=h, qh=qh, kh=kh, vt=vt, ah=ah,
                        pv_banks=[None] * 4, roll=roll,
                    )

                def strip_ap(st, jb, c0, c1):
                    base = (jb % 8) * 640
                    return st["roll"][:, base + c0 : base + c1]

                def emit_scores(st, i):
                    """Scores + mask + exp + 1/rowsum diag for block i."""
                    qh, kh = st["qh"], st["kh"]
                    jlo = max(0, i * 128 - WINDOW)
                    w = i * 128 + 128 - jlo
                    qb = qh[:, i * 128 : (i + 1) * 128]
                    ps_s = pss_pool.tile([128, 1024], f32, tag="pss")
                    if i >= 4:
                        # bank 0: edge-mask preload + 128-wide edge scores,
                        # then the 384-wide unmasked middle (pending-zero)
                        nc.tensor.matmul(
                            ps_s[:, 0:128], idnb[:], medge[:],
                            start=True, stop=False, skip_group_check=True,
                        )
                        nc.tensor.matmul(
                            ps_s[:, 128:512],
                            qb, kh[:, jlo + 128 : jlo + 512],
                            start=False, stop=False, skip_group_check=True,
                        )
                        nc.tensor.matmul(
                            ps_s[:, 0:128],
                            qb, kh[:, jlo : jlo + 128],
                            start=False, stop=True, skip_group_check=True,
                        )
                        # bank 1: diagonal block
                        nc.tensor.matmul(
                            ps_s[:, 512:640], idnb[:], mdiag[:],
                            start=True, stop=False, skip_group_check=True,
                        )
                        nc.tensor.matmul(
                            ps_s[:, 512:640],
                            qb, kh[:, jlo + 512 : jlo + 640],
                            start=False, stop=True, skip_group_check=True,
                        )
                    else:
                        dcol = w - 128
                        nc.tensor.matmul(
                            ps_s[:, dcol:w], idnb[:], mdiag[:],
                            start=True, stop=False, skip_group_check=True,
                        )
                        if dcol > 0:
                            nc.tensor.matmul(
                                ps_s[:, 0:dcol],
                                qb, kh[:, 0:dcol],
                                start=False, stop=False, skip_group_check=True,
                            )
                        nc.tensor.matmul(
                            ps_s[:, dcol:w],
                            qb, kh[:, dcol:w],
                            start=False, stop=True, skip_group_check=True,
                        )
                    pm = pm_pool.tile([128, 640], f16, tag="pm")
                    sums = sm_pool.tile([128, 1], f32, tag="sums")
                    nc.scalar.activation(
                        pm[:, :w], ps_s[:, :w], AF.Exp,
                        bias=ebias[:], accum_out=sums[:],
                    )
                    rc = sm_pool.tile([128, 1], f32, tag="rc")
                    nc.vector.reciprocal(rc[:], sums[:])
                    # normalize in fp16 (2-byte packed operands -> 2x DVE)
                    pr = pr_pool.tile([128, 640], f16, tag="pr")
                    nc.vector.tensor_scalar_mul(pr[:, :w], pm[:, :w], rc[:])
                    st["pms"][i] = pr

                def emit_tail(st, i):
                    """Transpose of normalized probs into strips + PV for i."""
                    h, pr = st["h"], st["pms"][i]
                    jlo = max(0, i * 128 - WINDOW)
                    w = i * 128 + 128 - jlo
                    nblk = w // 128
                    j0 = jlo // 128
                    # PE transpose with f16 psum output (1 bank for 640 cols)
                    ps_t = pst_pool.tile([128, 1024], f16, tag="pst")
                    for z in range(nblk):
                        nc.tensor.matmul(
                            ps_t[:, z * 128 : (z + 1) * 128],
                            pr[:, z * 128 : (z + 1) * 128],
                            idnh[:],
                            is_transpose=True,
                            start=(z == 0),
                            stop=(z == nblk - 1),
                            skip_group_check=True,
                        )
                    roll = st["roll"]
                    # dest col for z: ((j0+z)%8)*640 + (i-j0-z)*128 advances by
                    # 512 per z within a non-wrapping slot segment -> axis-
                    # aligned in a (col/512, (col%512)/128) view.
                    roll4 = roll[:].rearrange("p (a b c) -> p a b c", b=4, c=128)
                    ps4 = ps_t[:, : nblk * 128].rearrange(
                        "p (z o c) -> p z o c", o=1, c=128
                    )
                    z = 0
                    while z < nblk:
                        s0 = (j0 + z) % 8
                        zlen = min(nblk - z, 8 - s0)
                        base = s0 * 640 + (i - j0 - z) * 128
                        a0, b0 = base // 512, (base % 512) // 128
                        nc.vector.tensor_copy(
                            roll4[:, a0 : a0 + zlen, b0 : b0 + 1, :],
                            ps4[:, z : z + zlen, :, :],
                        )
                        z += zlen
                    # PV pieces that become ready once block i's strips exist
                    vt, ah = st["vt"], st["ah"]
                    pv_banks = st["pv_banks"]
                    for jb, c, nxt, pstart, pstop in PIECES.get(i, ()):
                        bk = c // 512
                        if pv_banks[bk] is None:
                            pv_banks[bk] = pso_pool.tile(
                                [128, 512], f32, tag="pvo",
                                name=f"pvo_h{h}_b{bk}",
                            )
                        nc.tensor.matmul(
                            pv_banks[bk][:, c - bk * 512 : nxt - bk * 512],
                            vt[jb],
                            strip_ap(st, jb, c - jb * 128, nxt - jb * 128),
                            start=pstart,
                            stop=pstop,
                            skip_group_check=True,
                        )
                        if pstop:
                            nc.vector.tensor_copy(
                                ah[:, bk * 512 : (bk + 1) * 512],
                                pv_banks[bk][:],
                            )

                for h in range(HPC):
                    st = setup_head(h)
                    st["pms"] = {}
                    if cfg["pipelined"]:
                        for i in range(NST):
                            emit_scores(st, i)
                            if i > 0:
                                emit_tail(st, i - 1)
                        emit_tail(st, NST - 1)
                    else:
                        for i in range(NST):
                            emit_scores(st, i)
                            emit_tail(st, i)

                phps.close()
                if cfg["merge_bc"] and "C" in cfg["phases"]:
                    emit_phase_c(ph)

            # ------------- Phase C: output projection (standalone) -------------
            if "C" in cfg["phases"] and not cfg["merge_bc"]:
                with ExitStack() as ph:
                    emit_phase_c(ph)

    nc.compile()
    return nc


def _get_module(repeat=1, cfg=None):
    _enable_ldw_opt()
    key = ("nc", repeat, tuple(sorted((cfg or {}).items())))
    if key not in _CACHE:
        _CACHE[key] = _build_module(repeat, cfg)
    return _CACHE[key]


def make_in_maps(hidden_states, cos, sin, w_qkv, w_o):
    hidden_states = np.asarray(hidden_states, dtype=np.float32)
    cos = np.asarray(cos, dtype=np.float32)
    sin = np.asarray(sin, dtype=np.float32)
    w_qkv = np.asarray(w_qkv, dtype=np.float32)
    w_o = np.asarray(w_o, dtype=np.float32)

    cosT = np.ascontiguousarray(cos.T)  # [DH, S]
    sinT = np.ascontiguousarray(sin.T)
    sinS = sinT.copy()
    sinS[: DH // 2] *= -1.0  # fold rotate_half sign
    cq = (cosT * SCALE).astype(np.float16)
    sq = (sinS * SCALE).astype(np.float16)
    ck = cosT.astype(np.float16)
    sk = sinS.astype(np.float16)

    import ml_dtypes

    rr = np.arange(128)[:, None]
    cc = np.arange(128)[None, :]
    # diagonal block (jb == i): allow j <= q  <->  c <= r
    mdiag = np.where(cc <= rr, 0.0, NEG).astype(ml_dtypes.bfloat16)
    # window-edge block (jb == i-4): allow q-j < 512  <->  c > r
    medge = np.where(cc > rr, 0.0, NEG).astype(ml_dtypes.bfloat16)
    idn = np.eye(128, dtype=np.float16)
    idnb = np.eye(128, dtype=ml_dtypes.bfloat16)

    xTs = [np.ascontiguousarray(hidden_states[b].T).astype(np.float16) for b in range(B)]

    in_maps = []
    for c in range(N_CORES):
        b, hg = divmod(c, N_CORES // B)
        r0 = hg * HPC * DH
        wq = w_qkv[r0 : r0 + HPC * DH]
        wk = w_qkv[N_HEADS * DH + r0 : N_HEADS * DH + r0 + HPC * DH]
        wv = w_qkv[2 * N_HEADS * DH + r0 : 2 * N_HEADS * DH + r0 + HPC * DH]
        wTqk = np.concatenate([wq, wk, wv], axis=0).T.astype(np.float16)  # [2048, 1536]
        # swizzle: wTq_sw[p, t_o*HIDDEN + k*128 + m] = wTqk[k*128+p, t_o*128+m]
        w4 = wTqk.reshape(HIDDEN // 128, 128, 3 * HPC, DH)  # [k, p, t_o, m]
        wTq_sw = np.ascontiguousarray(
            w4.transpose(1, 2, 0, 3).reshape(128, 3 * HPC * HIDDEN)
        )
        woTc = np.ascontiguousarray(w_o[:, r0 : r0 + HPC * DH].T).astype(np.float16)
        in_maps.append(
            {
                "xT": xTs[b],
                "wTq": wTq_sw,
                "woT": woTc,
                "cosq": cq,
                "sinq": sq,
                "cosk": ck,
                "sink": sk,
                "mdiag": mdiag,
                "medge": medge,
                "idn": idn,
                "idnb": idnb,
            }
        )
    return in_maps


def gather(results):
    out = np.zeros((B, S, HIDDEN), dtype=np.float32)
    for c in range(N_CORES):
        b = c // (N_CORES // B)
        out[b] += results[c]["out"].astype(np.float32)
    return out


def kernel(hidden_states, cos, sin, w_qkv, w_o):
    from concourse.bass_utils import run_bass_kernel_spmd

    nc = _get_module()
    in_maps = make_in_maps(hidden_states, cos, sin, w_qkv, w_o)
    res = run_bass_kernel_spmd(nc, in_maps, list(range(N_CORES)))
    return gather(res.results)


# revision 88
# speedup vs baseline: 1.0502x; 1.0085x over previous
"""Sliding-window attention (RoPE + QKV proj + windowed softmax attention + o_proj)
for Trainium2, SPMD over 8 NeuronCores.

Sharding: batch (2) x head-groups (4 groups of 4 heads) -> 8 cores.
Each core computes qkv for its 4 heads, windowed attention, and a partial
o_proj (its heads' columns of w_o); host sums the 4 partials per batch.

Datapath is fp16 (x, w, q/k/v, probs, wo) with fp32 PSUM accumulation:
fp16 matmuls run 1 cycle/row at any moving width (f32r needs >=256), and
halve DMA traffic. Softmax normalization is folded into the P->P^T
transpose by multiplying with diag(1/rowsum) instead of the identity.
Only the diagonal and window-edge 128-blocks get a mask preload; all
other score pieces start their psum bank directly (start=True zeros the
whole 2KB bank).
"""
import sys

sys.path.insert(0, "/opt/trn_rl_repo")

import numpy as np

B = 2
S = 2048
HIDDEN = 2048
N_HEADS = 16
DH = 128
WINDOW = 512
HPC = 4  # heads per core
N_CORES = 8
QKV_O = 3 * HPC * DH  # 1536
SCALE = 1.0 / np.sqrt(DH)
NEG = -1.0e30
EXP_BIAS = -3.0  # exp(s + bias): keeps fp16 probs well inside range

_CACHE = {}

CFG = {
    "psq_bufs": 8, "psv_bufs": 2,
    "xt_extra": 14, "rope_bufs": 2, "evac_bufs": 3, "tbl_bufs": 2,
    "pm_bufs": 3, "pr_bufs": 3, "v_bufs": 32, "roll_bufs": 2,
    "sm_bufs": 8,
    "pss_bufs": 2, "pst_bufs": 2, "pso_bufs": 2,
    "psc_bufs": 4, "ob_bufs": 6,
    "phases": "ABC", "merge_bc": True,
    "spill_eng": "sync", "out_eng": "scalar",
    "pipelined": True,
}


def _enable_ldw_opt():
    """walrus crashes with --enable-ldw-opt=true; keep the default."""
    return


def _build_module(repeat=1, cfg=None):
    cfg = {**CFG, **(cfg or {})}
    import concourse.tile as tile
    from concourse import bacc, mybir
    from contextlib import ExitStack

    f32 = mybir.dt.float32
    f16 = mybir.dt.float16
    bf16 = mybir.dt.bfloat16
    AF = mybir.ActivationFunctionType

    nc = bacc.Bacc("TRN2", target_bir_lowering=False, debug=False)

    xT = nc.declare_dram_parameter("xT", [HIDDEN, S], f16, isOutput=False)
    # qkv weights pre-swizzled per 128-col output group: col t_o*HIDDEN +
    # k*128 + m holds wT[k*128+p, t_o*128+m] -> one contiguous DMA per group
    wTq = nc.declare_dram_parameter(
        "wTq", [128, 3 * HPC * HIDDEN], f16, isOutput=False
    )
    woT = nc.declare_dram_parameter("woT", [HPC * DH, HIDDEN], f16, isOutput=False)
    cosq = nc.declare_dram_parameter("cosq", [DH, S], f16, isOutput=False)
    sinq = nc.declare_dram_parameter("sinq", [DH, S], f16, isOutput=False)
    cosk = nc.declare_dram_parameter("cosk", [DH, S], f16, isOutput=False)
    sink = nc.declare_dram_parameter("sink", [DH, S], f16, isOutput=False)
    mdiag_d = nc.declare_dram_parameter("mdiag", [128, 128], bf16, isOutput=False)
    medge_d = nc.declare_dram_parameter("medge", [128, 128], bf16, isOutput=False)
    idnb_d = nc.declare_dram_parameter("idnb", [128, 128], bf16, isOutput=False)
    idn_d = nc.declare_dram_parameter("idn", [128, 128], f16, isOutput=False)
    out_d = nc.declare_dram_parameter("out", [S, HIDDEN], f16, isOutput=True)

    NKT = HIDDEN // 128  # 16 contraction chunks
    NSC = S // 512  # 4 sequence chunks
    NST = S // 128  # 16 sequence tiles

    with tile.TileContext(nc) as tc, ExitStack() as top:
        dram = top.enter_context(tc.tile_pool(name="dram", bufs=1, space="DRAM"))
        v_sdT = dram.tile([HPC * DH, S], f16, tag="vsdT")

        cpool = top.enter_context(tc.tile_pool(name="consts", bufs=1))
        mdiag = cpool.tile([128, 128], bf16, tag="mdiag")
        nc.sync.dma_start(mdiag[:], mdiag_d[:])
        medge = cpool.tile([128, 128], bf16, tag="medge")
        nc.sync.dma_start(medge[:], medge_d[:])
        idnb = cpool.tile([128, 128], bf16, tag="idnb")
        nc.sync.dma_start(idnb[:], idnb_d[:])
        idnh = cpool.tile([128, 128], f16, tag="idnh")
        nc.sync.dma_start(idnh[:], idn_d[:])
        ebias = cpool.tile([128, 1], f32, tag="ebias")
        nc.vector.memset(ebias[:], EXP_BIAS)

        attn_pool = top.enter_context(tc.tile_pool(name="attn", bufs=HPC))
        # rope'd q/k stay in SBUF between phases A and B; double-buffered
        # (2 reps x 8 tiles) so repeat n+1's phase A never waits on repeat
        # n's phase B reads. v blocks reload via DMA-transpose during A.
        qkv_pool = top.enter_context(tc.tile_pool(name="qkv", bufs=4 * HPC))
        v_pool = top.enter_context(tc.tile_pool(name="vt", bufs=CFG["v_bufs"]))

        for rep in range(repeat):
            qk_tiles = [
                qkv_pool.tile([128, S], f16, tag="qk", name=f"qk{rep}_{t}")
                for t in range(2 * HPC)
            ]
            vblocks = [
                v_pool.tile([128, HPC * DH], f16, tag="vb", name=f"vb{rep}_{j}")
                for j in range(NST)
            ]
            # ------------- Phase A: QKV projection + RoPE -------------
            if "A" in cfg["phases"]:
              with ExitStack() as ph:
                wt_pool = ph.enter_context(tc.tile_pool(name="wt", bufs=NKT))
                xt_pool = ph.enter_context(tc.tile_pool(name="xt", bufs=NKT + cfg["xt_extra"]))
                tbl_pool = ph.enter_context(tc.tile_pool(name="tbl", bufs=cfg["tbl_bufs"]))
                rope_pool = ph.enter_context(tc.tile_pool(name="rope", bufs=cfg["rope_bufs"]))
                evac_pool = ph.enter_context(tc.tile_pool(name="evac", bufs=cfg["evac_bufs"]))
                psq_pool = ph.enter_context(
                    tc.tile_pool(name="psq", bufs=cfg["psq_bufs"], space="PSUM")
                )

                # q/k weights as [128,128] column tiles, group-major, so the
                # first projection group's weights land in ~2us instead of
                # waiting for full [128, QKV_O] rows; v weights as 512-wide
                # moving tiles. xt goes on the Activation hwdge queue, wt on
                # the SP queue -- the two streams run in parallel.
                # All loads go on the SP hwdge queue: the Activation queue
                # must stay empty so phase B's exp chain isn't serialized
                # behind load dispatches (engine queues are in-order).
                # Emission order puts group 0's inputs first.
                def load_wtq(t_o):
                    t = wt_pool.tile(
                        [128, HIDDEN], f16, tag="wtq", bufs=3 * HPC
                    )
                    nc.sync.dma_start(
                        t[:], wTq[:, t_o * HIDDEN : (t_o + 1) * HIDDEN]
                    )
                    return t

                wtq = [load_wtq(0)]

                def load_xt(sc):
                    s0 = sc * 512
                    xt_tiles = []
                    for k in range(NKT):
                        t = xt_pool.tile([128, 512], f16, tag="xt")
                        nc.sync.dma_start(
                            t[:], xT[k * 128 : (k + 1) * 128, s0 : s0 + 512]
                        )
                        xt_tiles.append(t)
                    return xt_tiles

                def load_tbl(sc):
                    s0 = sc * 512
                    tb = {}
                    for nm, src in (
                        ("cosq", cosq),
                        ("sinq", sinq),
                        ("cosk", cosk),
                        ("sink", sink),
                    ):
                        t = tbl_pool.tile([128, 512], f16, tag=nm)
                        nc.sync.dma_start(t[:], src[:, s0 : s0 + 512])
                        tb[nm] = t
                    return tb

                def load_xt_tbl(sc):
                    return load_xt(sc), load_tbl(sc)

                # arrival deadlines: group g's weights at ~3.4us*g, all xt
                # within group 0, tables only by the first rope -> order
                # wtq0, xt, wtq1, tbl, rest
                xt0 = load_xt(0)
                wtq.append(load_wtq(1))
                tb0 = load_tbl(0)
                wtq += [load_wtq(t_o) for t_o in range(2, 3 * HPC)]

                for sc in range(NSC):
                    s0 = sc * 512
                    xt_tiles, tb = (xt0, tb0) if sc == 0 else load_xt_tbl(sc)

                    def rope_evac(t_o, ps):
                        ct = tb["cosq"] if t_o < HPC else tb["cosk"]
                        st = tb["sinq"] if t_o < HPC else tb["sink"]
                        tmp = rope_pool.tile([128, 512], f16, tag="tmp")
                        nc.vector.tensor_mul(tmp[0:64, :], ps[64:128, :], st[0:64, :])
                        nc.vector.tensor_mul(
                            tmp[64:128, :], ps[0:64, :], st[64:128, :]
                        )
                        qc = rope_pool.tile([128, 512], f16, tag="qc")
                        nc.vector.tensor_mul(qc[:], ps[:], ct[:])
                        nc.vector.tensor_add(
                            qk_tiles[t_o][:, s0 : s0 + 512], qc[:], tmp[:]
                        )

                    def v_evac(vg, psv):
                        # v in [vcol, seq] layout; spill then reload each
                        # 128-seq block via DMA-transpose as [seq, vcol]
                        vo = evac_pool.tile([128, 512], f16, tag="vo")
                        nc.vector.tensor_copy(vo[:], psv[:])
                        getattr(nc, cfg["spill_eng"]).dma_start(
                            v_sdT[vg * 128 : (vg + 1) * 128, s0 : s0 + 512],
                            vo[:],
                        )

                    for t_o in range(3 * HPC):
                        ps = psq_pool.tile([128, 512], f32, tag="psq")
                        for k in range(NKT):
                            nc.tensor.matmul(
                                ps[:],
                                wtq[t_o][:, k * 128 : (k + 1) * 128],
                                xt_tiles[k][:],
                                start=(k == 0),
                                stop=(k == NKT - 1),
                            )
                        if t_o < 2 * HPC:
                            rope_evac(t_o, ps)
                        else:
                            v_evac(t_o - 2 * HPC, ps)
                    for jb in range(4 * sc, 4 * sc + 4):
                        nc.sync.dma_start_transpose(
                            vblocks[jb][:],
                            v_sdT[:, jb * 128 : (jb + 1) * 128],
                        )

            # ------------- Phase C body (emitted merged or standalone) ----
            attn_tiles = []

            def emit_phase_c(ph):
                ob_pool = ph.enter_context(
                    tc.tile_pool(name="ob", bufs=cfg["ob_bufs"])
                )
                psc_pool = ph.enter_context(
                    tc.tile_pool(name="psc", bufs=cfg["psc_bufs"], space="PSUM")
                )
                wo_pool = ph.enter_context(tc.tile_pool(name="wo", bufs=HPC))
                wts = []
                for h in range(HPC):
                    t = wo_pool.tile([128, HIDDEN], f16, tag="wo")
                    nc.sync.dma_start(t[:], woT[h * 128 : (h + 1) * 128, :])
                    wts.append(t)
                # h middle / mc inner: the stationary (attn piece) is reused
                # across the 4 mc matmuls, so the hardware reloads weights
                # once per (st_i, h) instead of per matmul.
                for st_i in range(NST):
                    pss4 = [
                        psc_pool.tile([128, 512], f32, tag="psc", name=f"psc{mc}")
                        for mc in range(HIDDEN // 512)
                    ]
                    for h in range(HPC):
                        for mc in range(HIDDEN // 512):
                            nc.tensor.matmul(
                                pss4[mc][:],
                                attn_tiles[h][:, st_i * 128 : (st_i + 1) * 128],
                                wts[h][:, mc * 512 : (mc + 1) * 512],
                                start=(h == 0),
                                stop=(h == HPC - 1),
                            )
                    for mc in range(HIDDEN // 512):
                        ob = ob_pool.tile([128, 512], f16, tag="ob")
                        # alternate evacs between Act and the otherwise-idle
                        # DVE: Act alone is the phase-C throughput wall
                        if mc % 2 == 0:
                            nc.scalar.activation(ob[:], pss4[mc][:], AF.Copy)
                        else:
                            nc.vector.tensor_copy(ob[:], pss4[mc][:])
                        getattr(nc, cfg["out_eng"]).dma_start(
                            out_d[
                                st_i * 128 : (st_i + 1) * 128,
                                mc * 512 : (mc + 1) * 512,
                            ],
                            ob[:],
                        )

            # ------------- Phase B: windowed attention -------------
            if "B" in cfg["phases"]:
              with ExitStack() as ph:
                pm_pool = ph.enter_context(tc.tile_pool(name="pm", bufs=cfg["pm_bufs"]))
                pr_pool = ph.enter_context(tc.tile_pool(name="pr", bufs=cfg["pr_bufs"]))
                sm_pool = ph.enter_context(tc.tile_pool(name="sm", bufs=cfg["sm_bufs"]))
                strip_pool = ph.enter_context(
                    tc.tile_pool(name="strip", bufs=cfg["roll_bufs"])
                )
                phps = ExitStack()
                pss_pool = phps.enter_context(
                    tc.tile_pool(name="pss", bufs=cfg["pss_bufs"], space="PSUM")
                )
                pst_pool = phps.enter_context(
                    tc.tile_pool(name="pst", bufs=cfg["pst_bufs"], space="PSUM")
                )
                pso_pool = phps.enter_context(
                    tc.tile_pool(name="pso", bufs=cfg["pso_bufs"], space="PSUM")
                )

                # last q-block whose PV contribution lands in psum bank bk
                LAST_BANK = {0: 3, 1: 7, 2: 11, 3: 15}

                def make_pieces():
                    """PV pieces keyed by ready block, with start/stop flags.

                    Pieces are cut at psum bank boundaries and at each bank's
                    high-water mark so a single matmul never mixes
                    already-written and pending-zero bytes. The first piece
                    per bank carries start=True (zeros the whole bank)."""
                    hw = {}
                    by_ready = {}
                    for jb in range(NST):
                        w0, w1 = jb * 128, min(jb * 128 + 640, S)
                        c = w0
                        while c < w1:
                            bk = c // 512
                            nxt = min(w1, (bk + 1) * 512)
                            first = bk not in hw
                            hm = hw.get(bk, c)
                            if not first and c < hm < nxt:
                                subs = [(c, hm), (hm, nxt)]
                            else:
                                subs = [(c, nxt)]
                            for a, b in subs:
                                by_ready.setdefault(min(jb + 4, NST - 1), []).append(
                                    (
                                        jb,
                                        a,
                                        b,
                                        first and a == c,
                                        jb == LAST_BANK[bk] and b == nxt,
                                    )
                                )
                            hw[bk] = max(hm, nxt)
                            c = nxt
                    return by_ready

                PIECES = make_pieces()

                def setup_head(h):
                    qh = qk_tiles[h]
                    kh = qk_tiles[HPC + h]
                    vt = [vblocks[jb][:, h * 128 : (h + 1) * 128] for jb in range(NST)]
                    ah = attn_pool.tile([128, S], f16, tag="ah", name=f"ah{h}")
                    attn_tiles.append(ah)
                    roll = strip_pool.tile(
                        [128, 8 * 640], f16, tag="roll", name=f"roll{h}"
                    )
                    return dict(
                        h=h, qh=qh, kh=kh, vt=vt, ah=ah,
                        pv_banks=[None] * 4, roll=roll,
                    )

                def strip_ap(st, jb, c0, c1):
                    base = (jb % 8) * 640
                    return st["roll"][:, base + c0 : base + c1]

                def emit_scores(st, i):
                    """Scores + mask + exp + 1/rowsum diag for block i."""
                    qh, kh = st["qh"], st["kh"]
                    jlo = max(0, i * 128 - WINDOW)
                    w = i * 128 + 128 - jlo
                    qb = qh[:, i * 128 : (i + 1) * 128]
                    ps_s = pss_pool.tile([128, 1024], f32, tag="pss")
                    if i >= 4:
                        # bank 0: edge-mask preload + 128-wide edge scores,
                        # then the 384-wide unmasked middle (pending-zero)
                        nc.tensor.matmul(
                            ps_s[:, 0:128], idnb[:], medge[:],
                            start=True, stop=False, skip_group_check=True,
                        )
                        nc.tensor.matmul(
                            ps_s[:, 128:512],
                            qb, kh[:, jlo + 128 : jlo + 512],
                            start=False, stop=False, skip_group_check=True,
                        )
                        nc.tensor.matmul(
                            ps_s[:, 0:128],
                            qb, kh[:, jlo : jlo + 128],
                            start=False, stop=True, skip_group_check=True,
                        )
                        # bank 1: diagonal block
                        nc.tensor.matmul(
                            ps_s[:, 512:640], idnb[:], mdiag[:],
                            start=True, stop=False, skip_group_check=True,
                        )
                        nc.tensor.matmul(
                            ps_s[:, 512:640],
                            qb, kh[:, jlo + 512 : jlo + 640],
                            start=False, stop=True, skip_group_check=True,
                        )
                    else:
                        dcol = w - 128
                        nc.tensor.matmul(
                            ps_s[:, dcol:w], idnb[:], mdiag[:],
                            start=True, stop=False, skip_group_check=True,
                        )
                        if dcol > 0:
                            nc.tensor.matmul(
                                ps_s[:, 0:dcol],
                                qb, kh[:, 0:dcol],
                                start=False, stop=False, skip_group_check=True,
                            )
                        nc.tensor.matmul(
                            ps_s[:, dcol:w],
                            qb, kh[:, dcol:w],
                            start=False, stop=True, skip_group_check=True,
                        )
                    pm = pm_pool.tile([128, 640], f16, tag="pm")
                    sums = sm_pool.tile([128, 1], f32, tag="sums")
                    nc.scalar.activation(
                        pm[:, :w], ps_s[:, :w], AF.Exp,
                        bias=ebias[:], accum_out=sums[:],
                    )
                    rc = sm_pool.tile([128, 1], f32, tag="rc")
                    nc.vector.reciprocal(rc[:], sums[:])
                    # normalize in fp16 (2-byte packed operands -> 2x DVE)
                    pr = pr_pool.tile([128, 640], f16, tag="pr")
                    nc.vector.tensor_scalar_mul(pr[:, :w], pm[:, :w], rc[:])
                    st["pms"][i] = pr

                def emit_tail(st, i):
                    """Transpose of normalized probs into strips + PV for i."""
                    h, pr = st["h"], st["pms"][i]
                    jlo = max(0, i * 128 - WINDOW)
                    w = i * 128 + 128 - jlo
                    nblk = w // 128
                    j0 = jlo // 128
                    # PE transpose with f16 psum output (1 bank for 640 cols)
                    ps_t = pst_pool.tile([128, 1024], f16, tag="pst")
                    for z in range(nblk):
                        nc.tensor.matmul(
                            ps_t[:, z * 128 : (z + 1) * 128],
                            pr[:, z * 128 : (z + 1) * 128],
                            idnh[:],
                            is_transpose=True,
                            start=(z == 0),
                            stop=(z == nblk - 1),
                            skip_group_check=True,
                        )
                    roll = st["roll"]
                    # dest col for z: ((j0+z)%8)*640 + (i-j0-z)*128 advances by
                    # 512 per z within a non-wrapping slot segment -> axis-
                    # aligned in a (col/512, (col%512)/128) view.
                    roll4 = roll[:].rearrange("p (a b c) -> p a b c", b=4, c=128)
                    ps4 = ps_t[:, : nblk * 128].rearrange(
                        "p (z o c) -> p z o c", o=1, c=128
                    )
                    z = 0
                    while z < nblk:
                        s0 = (j0 + z) % 8
                        zlen = min(nblk - z, 8 - s0)
                        base = s0 * 640 + (i - j0 - z) * 128
                        a0, b0 = base // 512, (base % 512) // 128
                        nc.vector.tensor_copy(
                            roll4[:, a0 : a0 + zlen, b0 : b0 + 1, :],
                            ps4[:, z : z + zlen, :, :],
                        )
                        z += zlen
                    # PV pieces that become ready once block i's strips exist
                    vt, ah = st["vt"], st["ah"]
                    pv_banks = st["pv_banks"]
                    for jb, c, nxt, pstart, pstop in PIECES.get(i, ()):
                        bk = c // 512
                        if pv_banks[bk] is None:
                            pv_banks[bk] = pso_pool.tile(
                                [128, 512], f32, tag="pvo",
                                name=f"pvo_h{h}_b{bk}",
                            )
                        nc.tensor.matmul(
                            pv_banks[bk][:, c - bk * 512 : nxt - bk * 512],
                            vt[jb],
                            strip_ap(st, jb, c - jb * 128, nxt - jb * 128),
                            start=pstart,
                            stop=pstop,
                            skip_group_check=True,
                        )
                        if pstop:
                            nc.vector.tensor_copy(
                                ah[:, bk * 512 : (bk + 1) * 512],
                                pv_banks[bk][:],
                            )

                for h in range(HPC):
                    st = setup_head(h)
                    st["pms"] = {}
                    if cfg["pipelined"]:
                        for i in range(NST):
                            emit_scores(st, i)
                            if i > 0:
                                emit_tail(st, i - 1)
                        emit_tail(st, NST - 1)
                    else:
                        for i in range(NST):
                            emit_scores(st, i)
                            emit_tail(st, i)

                phps.close()
                if cfg["merge_bc"] and "C" in cfg["phases"]:
                    emit_phase_c(ph)

            # ------------- Phase C: output projection (standalone) -------------
            if "C" in cfg["phases"] and not cfg["merge_bc"]:
                with ExitStack() as ph:
                    emit_phase_c(ph)

    nc.compile()
    return nc


def _get_module(repeat=1, cfg=None):
    _enable_ldw_opt()
    key = ("nc", repeat, tuple(sorted((cfg or {}).items())))
    if key not in _CACHE:
        _CACHE[key] = _build_module(repeat, cfg)
    return _CACHE[key]


def make_in_maps(hidden_states, cos, sin, w_qkv, w_o):
    hidden_states = np.asarray(hidden_states, dtype=np.float32)
    cos = np.asarray(cos, dtype=np.float32)
    sin = np.asarray(sin, dtype=np.float32)
    w_qkv = np.asarray(w_qkv, dtype=np.float32)
    w_o = np.asarray(w_o, dtype=np.float32)

    cosT = np.ascontiguousarray(cos.T)  # [DH, S]
    sinT = np.ascontiguousarray(sin.T)
    sinS = sinT.copy()
    sinS[: DH // 2] *= -1.0  # fold rotate_half sign
    cq = (cosT * SCALE).astype(np.float16)
    sq = (sinS * SCALE).astype(np.float16)
    ck = cosT.astype(np.float16)
    sk = sinS.astype(np.float16)

    import ml_dtypes

    rr = np.arange(128)[:, None]
    cc = np.arange(128)[None, :]
    # diagonal block (jb == i): allow j <= q  <->  c <= r
    mdiag = np.where(cc <= rr, 0.0, NEG).astype(ml_dtypes.bfloat16)
    # window-edge block (jb == i-4): allow q-j < 512  <->  c > r
    medge = np.where(cc > rr, 0.0, NEG).astype(ml_dtypes.bfloat16)
    idn = np.eye(128, dtype=np.float16)
    idnb = np.eye(128, dtype=ml_dtypes.bfloat16)

    xTs = [np.ascontiguousarray(hidden_states[b].T).astype(np.float16) for b in range(B)]

    in_maps = []
    for c in range(N_CORES):
        b, hg = divmod(c, N_CORES // B)
        r0 = hg * HPC * DH
        wq = w_qkv[r0 : r0 + HPC * DH]
        wk = w_qkv[N_HEADS * DH + r0 : N_HEADS * DH + r0 + HPC * DH]
        wv = w_qkv[2 * N_HEADS * DH + r0 : 2 * N_HEADS * DH + r0 + HPC * DH]
        wTqk = np.concatenate([wq, wk, wv], axis=0).T.astype(np.float16)  # [2048, 1536]
        # swizzle: wTq_sw[p, t_o*HIDDEN + k*128 + m] = wTqk[k*128+p, t_o*128+m]
        w4 = wTqk.reshape(HIDDEN // 128, 128, 3 * HPC, DH)  # [k, p, t_o, m]
        wTq_sw = np.ascontiguousarray(
            w4.transpose(1, 2, 0, 3).reshape(128, 3 * HPC * HIDDEN)
        )
        woTc = np.ascontiguousarray(w_o[:, r0 : r0 + HPC * DH].T).astype(np.float16)
        in_maps.append(
            {
                "xT": xTs[b],
                "wTq": wTq_sw,
                "woT": woTc,
                "cosq": cq,
                "sinq": sq,
                "cosk": ck,
                "sink": sk,
                "mdiag": mdiag,
                "medge": medge,
                "idn": idn,
                "idnb": idnb,
            }
        )
    return in_maps


def gather(results):
    out = np.zeros((B, S, HIDDEN), dtype=np.float32)
    for c in range(N_CORES):
        b = c // (N_CORES // B)
        out[b] += results[c]["out"].astype(np.float32)
    return out


def kernel(hidden_states, cos, sin, w_qkv, w_o):
    from concourse.bass_utils import run_bass_kernel_spmd

    nc = _get_module()
    in_maps = make_in_maps(hidden_states, cos, sin, w_qkv, w_o)
    res = run_bass_kernel_spmd(nc, in_maps, list(range(N_CORES)))
    return gather(res.results)
